# revision 9
# baseline (speedup 1.0000x reference)
"""Correlation1D Trainium2 Bass kernel.

out[b, d, h, w] = (1/C) * sum_c in1[b, c, h, w] * in2pad[b, c, h, w + d]
  B=8, C=256, H=96, W=192, PAD=40, D=81 displacement channels.

Strategy (data-parallel over batch, 1 sample per NeuronCore):
  For each h row and each w-chunk of CHUNK, compute a Gram band
      G[w, e] = sum_c in1[c, w] * in2pad[c, w0 + e]   (PE matmuls, k=c)
  with e in [0, CHUNK + D - 1): exactly the band the 81 diagonals
  O[d, w] = G[w, w + d] live in.  Diagonals can't be extracted by any
  on-chip AP (partition/free strides are independent), so the band is
  written to DRAM scratch where a *flat* strided access pattern CAN walk
  diagonals: a gather DMA with partition stride (row_stride + 1) yields
  T[w, d] = G[w, w + d].  A PE transpose then gives O[d, w] tiles which
  are written out in the final [d, h, w] layout.

  Inputs are cast fp32 -> fp16 by the SWDGE loads; fp16 matmuls run at
  1 cycle/row at ANY moving width (unlike fp32r, which needs >=256), so
  the moving operand is just the CHUNK+80 band window — which is what
  lets CHUNK shrink and with it the DRAM scratch traffic.  The band is
  stored fp16 (values pre-scaled by 1/C; ~5e-4 rel err).
"""

import os

import numpy as np

import bass_rust as _br
import concourse.bass as bass
import concourse.tile as tile
from concourse import bacc, mybir
from concourse.bass_utils import run_bass_kernel_spmd

# Problem constants (hardcoded per harness contract)
B = 8
C = 256
H = 96
W = 192
PAD = 40
D = 2 * PAD + 1  # 81
W2 = W + 2 * PAD  # 272 padded width
CH = 2  # c is split into CH partition-halves of 128
CP = C // CH  # 128

# Tunables (env-overridable for experiments)
CHUNK = int(os.environ.get("CORR_CHUNK", "48"))  # w-chunk (Gram partition dim)
NCK = W // CHUNK
BANDW = CHUNK + D - 1  # band window width per chunk
HB = int(os.environ.get("CORR_HB", "8"))  # h rows per batch
NB = H // HB
BAND_DT_S = os.environ.get("CORR_BAND_DT", "fp16")  # fp32 | fp16 | bf16
MM_DT_S = os.environ.get("CORR_MM", "fp16")  # fp16 | bf16 | fp32r
IN_BUFS = int(os.environ.get("CORR_IN_BUFS", "3"))
G_BUFS = int(os.environ.get("CORR_G_BUFS", "3"))
PH = int(os.environ.get("CORR_PH", "2"))  # h rows batched per PSUM tile
GCK = 96 // CHUNK if CHUNK <= 96 else 1  # chunks per 96-wide gather/transpose

_DT = {
    "fp32": mybir.dt.float32,
    "fp16": mybir.dt.float16,
    "bf16": mybir.dt.bfloat16,
    "fp32r": mybir.dt.float32r,
}


def _build(reps=1):
    band_dt = _DT[BAND_DT_S]
    mm_dt = _DT[MM_DT_S]
    f32 = mybir.dt.float32

    nc = bacc.Bacc("TRN2")

    out_dt = _DT[OUT_DT_S]
    in1 = nc.dram_tensor("input1", [C, H, W], f32, kind="ExternalInput")
    in2 = nc.dram_tensor("input2", [C, H, W], f32, kind="ExternalInput")
    # fp16 out halves the final write; kernel() casts to fp32 host-side
    out = nc.dram_tensor("out", [D, H, W], out_dt, kind="ExternalOutput")
    # [wl, h, ck, e] so one ib's band leaves SBUF as a single DMA with
    # HB*NCK*BANDW-byte contiguous runs per partition
    scratch = nc.dram_tensor("scratch", [CHUNK, H, NCK, BANDW], band_dt)

    # [c, h, w] -> [p, a, h*w] so the in1 load is one 3-dim DMA
    in1_r = in1.ap().rearrange("(a p) h w -> p a (h w)", p=CP)
    in2_r = in2.ap().rearrange("(a p) h w -> p a h w", p=CP)
    out_ap = out.ap()
    scr_ap = scratch.ap()

    # casting loads (fp32 -> fp16 rounding) need SWDGE; plain fp32 can
    # use the faster HWDGE path
    load_eng = nc.gpsimd if MM_DT_S != "fp32" else nc.sync

    with tile.TileContext(nc) as tc:
        with (
            tc.tile_pool(name="singles", bufs=1) as singles,
            tc.tile_pool(name="loads", bufs=IN_BUFS) as loads,
            tc.tile_pool(name="bands", bufs=2) as bands,
            tc.tile_pool(name="gats", bufs=2) as gats,
            tc.tile_pool(name="outs", bufs=2) as outs,
            tc.tile_pool(name="psg", bufs=G_BUFS, space="PSUM") as psg,
            tc.tile_pool(name="pso", bufs=2, space="PSUM") as pso,
        ):
            # identity for PE transposes
            ident = singles.tile([96, 96], band_dt)
            from concourse.masks import make_identity

            make_identity(nc, ident[:])

            for _rep in range(reps):
              for ib in range(NB):
                h0 = ib * HB

                in1_t = loads.tile([CP, CH, HB, W], mm_dt)
                load_eng.dma_start(
                    out=in1_t[:].rearrange("p a h w -> p a (h w)"),
                    in_=in1_r[:, :, h0 * W : (h0 + HB) * W],
                )
                # in2 is zero-padded to 272 cols; matmuls slice BANDW-wide
                # windows out of it
                in2_t = loads.tile([CP, CH, HB, W2], mm_dt)
                # memset doesn't accept fp32r — zero through an f32 view
                # (zero bits are dtype-invariant); fp16/bf16 are fine but
                # keep the bitcast uniform
                nc.gpsimd.memset(in2_t[:, :, :, 0:PAD].bitcast(f32), 0.0)
                nc.gpsimd.memset(in2_t[:, :, :, PAD + W : W2].bitcast(f32), 0.0)
                for a in range(CH):
                    load_eng.dma_start(
                        out=in2_t[:, a, :, PAD : PAD + W],
                        in_=in2_r[:, a, h0 : h0 + HB, :],
                    )

                band_t = bands.tile([CHUNK, HB, NCK, BANDW], band_dt)

                for hp in range(HB // PH):
                    g = psg.tile([CHUNK, PH, NCK, BANDW], f32)
                    for ph in range(PH):
                        hl = hp * PH + ph
                        for ck in range(NCK):
                            # moving window [w0 - PAD, w0 + CHUNK + PAD)
                            # in unpadded coords = [w0, w0+BANDW) padded
                            w0 = ck * CHUNK
                            for a in range(CH):
                                nc.tensor.matmul(
                                    g[:, ph, ck, :],
                                    in1_t[:, a, hl, w0 : w0 + CHUNK],
                                    in2_t[:, a, hl, w0 : w0 + BANDW],
                                    start=(a == 0),
                                    stop=(a == CH - 1),
                                )
                    # one batched 1/C scale (+ cast) for PH*NCK matmul tiles
                    nc.scalar.mul(
                        out=band_t[:, hp * PH : (hp + 1) * PH, :, :],
                        in_=g[:],
                        mul=1.0 / C,
                    )

                band_dma = nc.sync.dma_start(
                    out=scr_ap[:, h0 : h0 + HB, :, :],
                    in_=band_t[:],
                )

                # --- phase 2: skewed gather + transpose + writeout ---
                # gather tiles span 96 partitions = GCK chunks each
                gat_ts = []
                for gk in range(W // 96):
                    gat = gats.tile(
                        [96, HB, D], band_dt,
                        name=f"gat{gk}_{_rep}_{ib}", tag=f"gat{gk}",
                    )
                    row = H * NCK * BANDW  # stride of wl in scratch
                    # tile gk covers w in [96*gk, 96*gk+96); walk it in
                    # chunk-aligned segments (ck, wl0, cnt)
                    w = gk * 96
                    while w < gk * 96 + 96:
                        ck, wl0 = divmod(w, CHUNK)
                        cnt = min(CHUNK - wl0, gk * 96 + 96 - w)
                        # T[w, hl, d] = scr[wl0+j, h0+hl, ck, wl0+j + d]
                        skew = bass.AP(
                            tensor=scr_ap.tensor,
                            offset=wl0 * (row + 1)
                            + h0 * NCK * BANDW
                            + ck * BANDW,
                            ap=[
                                [row + 1, cnt],
                                [NCK * BANDW, HB],
                                [1, D],
                            ],
                        )
                        gi = nc.sync.dma_start(
                            out=gat[w - gk * 96 : w - gk * 96 + cnt],
                            in_=skew,
                        )
                        # Explicit RAW edge through DRAM scratch (belt &
                        # braces in case AP-overlap detection misses the
                        # skewed stride).
                        _br.add_dep_helper(
                            gi.ins, band_dma.ins, reason="scratch RAW"
                        )
                        w += cnt
                    gat_ts.append(gat)

                out_t = outs.tile([D, HB, W], f32)
                for hl in range(HB):
                    po = pso.tile([D, W], band_dt)
                    for gk in range(W // 96):
                        nc.tensor.transpose(
                            out=po[:, gk * 96 : (gk + 1) * 96],
                            in_=gat_ts[gk][:, hl, :],
                            identity=ident[:],
                        )
                    nc.vector.tensor_copy(out=out_t[:, hl, :], in_=po[:])
                nc.sync.dma_start(out=out_ap[:, h0 : h0 + HB, :], in_=out_t[:])

    nc.compile()
    return nc


_NC_CACHE = None


def run(input1, input2, trace=False, **spmd_kwargs):
    """Run on 8 NeuronCores; returns (out [B,D,H,W] fp32, BassKernelResults)."""
    global _NC_CACHE
    if _NC_CACHE is None:
        _NC_CACHE = _build()
    nc = _NC_CACHE

    input1 = np.ascontiguousarray(np.asarray(input1), dtype=np.float32)
    input2 = np.ascontiguousarray(np.asarray(input2), dtype=np.float32)
    assert input1.shape == (B, C, H, W) and input2.shape == (B, C, H, W)

    in_maps = [
        {"input1": input1[b], "input2": input2[b]} for b in range(B)
    ]
    res = run_bass_kernel_spmd(
        nc, in_maps, core_ids=list(range(B)), trace=trace, **spmd_kwargs
    )
    out = np.stack([res.results[b]["out"] for b in range(B)], axis=0)
    return out, res


def kernel(input1, input2):
    out, _ = run(input1, input2)
    return out
